# revision 3
# baseline (speedup 1.0000x reference)
"""AnyLoc VLAD (vq_codebook) Trainium2 kernel, 8-core data parallel.

Reference computation (per image, N=1024 patches, K=64 clusters, D=1536):
  descs_n = l2norm(query_descs)                 # row-normalize descriptors
  labels  = argmax_k(descs_n . l2norm(centers)) # hard assignment
  sum_d_k = sum_{n: label=k} descs_n            # per-cluster sum
  un_vlad = sum_d_k - count_k * centers_k
  vlad    = l2norm_rows(un_vlad); flatten; l2norm

Device strategy (per core, 4 images):
  - host pre-casts descriptors to bf16 and pre-transposes/normalizes the
    tiny codebook; argmax is invariant to the descriptor's own norm so the
    sims matmul uses raw (unnormalized) descriptors
  - per 128-patch chunk: DMA load natural [128,1536]bf16, xbar-DMA
    transpose to [128d,12,128n], ACT/DVE square-accum for norms,
    12 accumulating TensorE matmuls -> sims [128,64] in PSUM,
    DVE max + (sims>=max)*inv_norm -> scaled one-hot assign [128,64]bf16,
    TensorE aggregation: assign^T @ [descs | norm] accumulated over the
    image into PSUM [64, 1537] (sum_desc and counts in one group)
  - per image: un_vlad = sum_desc - counts*centers (fp32 centers), row
    norms, ones-matmul to broadcast the global norm, final scale, DMA out.
"""

import os
import sys

import numpy as np

for _p in ("/opt/trn_rl_repo", "/root/.axon_site/_ro/trn_rl_repo"):
    if os.path.isdir(_p) and _p not in sys.path:
        sys.path.insert(0, _p)

import ml_dtypes
import bass_rust
import concourse.bass as bass
import concourse.tile as tile
from concourse import mybir
from concourse.bass_utils import run_bass_kernel_spmd

B, N, K, D = 32, 1024, 64, 1536
NCORES = 8
IMGS = B // NCORES  # images per core
P = 128
NCH = N // P   # 8 patch chunks per image
DC = D // P    # 12 feature chunks
BF16 = mybir.dt.bfloat16
F32 = mybir.dt.float32
NP_BF16 = ml_dtypes.bfloat16
Alu = mybir.AluOpType
Act = mybir.ActivationFunctionType
EPS = 1e-12


def _patch_tile_drain():
    """This walrus build only accepts ONE sync wait per instruction; Tile's
    tail drain aggregates every outstanding semaphore wait onto a single
    Drain. Spread the waits across extra per-engine drains (all still
    before the end-of-kernel barrier, so semantics are unchanged)."""
    if getattr(tile.TileContext, "_vlad_drain_patched", False):
        return
    from concourse.vector_clock import ScopedClock

    def patched(self, tick_clock, wait_clock):
        nc = self.nc
        probe = nc.sync.drain()
        wait_clock.add_sem_waits(
            probe.ins, ScopedClock({None: tick_clock.global_clock})
        )
        si = probe.ins.sync_info
        waits = list(si.on_wait) if si is not None else []
        upds = list(si.on_update) if si is not None else []
        probe.ins.sync_info = bass_rust.SyncInfo(on_wait=waits[:1], on_update=upds)
        engines = [nc.scalar, nc.vector, nc.tensor, nc.gpsimd, nc.sync]
        for i, w in enumerate(waits[1:]):
            d = engines[i % len(engines)].drain()
            dsi = d.ins.sync_info
            du = list(dsi.on_update) if dsi is not None else []
            d.ins.sync_info = bass_rust.SyncInfo(on_wait=[w], on_update=du)
        nc.all_engine_barrier()
        popped = nc._tile_sem_poison_stack.pop()
        assert popped is self._sem_poison
        nc.clear_and_free_semaphores(list(self.sems.allocated().values()))
        nc.all_engine_barrier()

    tile.TileContext._drain_and_barrier = patched
    tile.TileContext._vlad_drain_patched = True


def _split_multi_waits(nc):
    """Walrus here accepts only one sync wait per instruction. Hoist surplus
    waits onto no-op carrier instructions inserted just before, on the same
    engine (safe: same engine executes in order, so all waits still complete
    before the original instruction issues)."""
    n_new = 0
    for _bbname, bassbb in list(nc.bb_map.items()):
        bb = bassbb.bb
        out = []
        changed = False
        for ins in bb.instructions:
            si = getattr(ins, "sync_info", None)
            waits = list(si.on_wait) if si is not None else []
            if len(waits) > 1:
                changed = True
                for w in waits[:-1]:
                    n_new += 1
                    nop = mybir.InstNoOp(
                        name=f"{ins.name}-wsplit{n_new}",
                        sync_info=mybir.SyncInfo(on_wait=[w], on_update=[]),
                        bass_nofuse=True,
                        engine=ins.engine,
                    )
                    nc.register_instruction(nop)
                    out.append(nop)
                ins.sync_info = bass_rust.SyncInfo(
                    on_wait=[waits[-1]], on_update=list(si.on_update)
                )
            out.append(ins)
        if changed:
            bb.instructions = out
    return n_new


def build_nc(imgs=IMGS, nch=NCH):
    """Build the per-core Bass graph. `imgs`/`nch` shrinkable for sim tests."""
    _patch_tile_drain()
    n_rows = imgs * nch * P
    nc = bass.Bass("TRN2", target_bir_lowering=False, debug=False)
    descs_e = nc.dram_tensor("descs", [n_rows, D], BF16, kind="ExternalInput")
    cnt_e = nc.dram_tensor("cnormt", [D, K], BF16, kind="ExternalInput")
    cen_e = nc.dram_tensor("centers", [K, D], F32, kind="ExternalInput")
    out_e = nc.dram_tensor("out", [imgs, K * D], F32, kind="ExternalOutput")

    with tile.TileContext(nc) as tc:
        from contextlib import ExitStack

        with ExitStack() as ctx:
            consts = ctx.enter_context(tc.tile_pool(name="consts", bufs=1))
            natp = ctx.enter_context(tc.tile_pool(name="nat", bufs=3))
            tspp = ctx.enter_context(tc.tile_pool(name="tsp", bufs=3))
            sqp = ctx.enter_context(tc.tile_pool(name="sq", bufs=2))
            smallp = ctx.enter_context(tc.tile_pool(name="small", bufs=6))
            asnp = ctx.enter_context(tc.tile_pool(name="asn", bufs=3))
            vladp = ctx.enter_context(tc.tile_pool(name="vlad", bufs=2))
            finp = ctx.enter_context(tc.tile_pool(name="fin", bufs=4))
            simsp = ctx.enter_context(tc.tile_pool(name="simsps", bufs=2, space="PSUM"))
            aggp = ctx.enter_context(tc.tile_pool(name="aggps", bufs=1, space="PSUM"))
            gpsp = ctx.enter_context(tc.tile_pool(name="gps", bufs=2, space="PSUM"))

            cnt_sb = consts.tile([P, DC, K], BF16)
            nc.sync.dma_start(
                out=cnt_sb, in_=cnt_e.ap().rearrange("(c p) k -> p c k", p=P)
            )
            cen_sb = consts.tile([K, D], F32)
            nc.sync.dma_start(out=cen_sb, in_=cen_e.ap())
            ones64 = consts.tile([K, K], F32)
            nc.vector.memset(ones64, 1.0)

            for b in range(imgs):
                agg_ps = aggp.tile([K, 3 * 512 + 1], F32)
                for ci in range(nch):
                    r0 = (b * nch + ci) * P
                    nat = natp.tile([P, D], BF16)
                    nc.sync.dma_start(out=nat, in_=descs_e.ap()[r0 : r0 + P, :])
                    tsp = tspp.tile([P, DC, P], BF16)
                    nc.scalar.dma_start(out=tsp, in_=nat[:], transpose=True)

                    sq = sqp.tile([P, D], BF16, tag="sq")
                    ss = smallp.tile([P, 1], F32, tag="ss")
                    if ci % 2 == 0:
                        nc.scalar.activation(sq, nat, Act.Square, accum_out=ss)
                    else:
                        nc.vector.scalar_tensor_tensor(
                            out=sq, in0=nat, scalar=1.0, in1=nat,
                            op0=Alu.mult, op1=Alu.mult, accum_out=ss,
                        )
                    nrm = smallp.tile([P, 1], F32, tag="nrm")
                    nc.scalar.sqrt(nrm, ss)
                    nrm16 = smallp.tile([P, 1], BF16, tag="nrm16")
                    nc.vector.tensor_copy(nrm16, nrm)
                    nrmx = smallp.tile([P, 1], F32, tag="nrmx")
                    nc.vector.tensor_scalar_max(nrmx, nrm, EPS)
                    inv = smallp.tile([P, 1], F32, tag="inv")
                    nc.vector.reciprocal(inv, nrmx)

                    sims = simsp.tile([P, K], F32)
                    for d in range(DC):
                        nc.tensor.matmul(
                            sims, lhsT=tsp[:, d, :], rhs=cnt_sb[:, d, :],
                            start=(d == 0), stop=(d == DC - 1),
                        )
                    mx = smallp.tile([P, 1], F32, tag="mx")
                    nc.vector.tensor_reduce(mx, sims, axis=mybir.AxisListType.X, op=Alu.max)
                    asn = asnp.tile([P, K], BF16)
                    nc.vector.tensor_scalar(
                        asn, sims, scalar1=mx, scalar2=inv, op0=Alu.is_ge, op1=Alu.mult
                    )

                    first, last = ci == 0, ci == nch - 1
                    for j in range(3):
                        nc.tensor.matmul(
                            agg_ps[:, j * 512 : (j + 1) * 512],
                            lhsT=asn, rhs=nat[:, j * 512 : (j + 1) * 512],
                            start=first, stop=last,
                        )
                    nc.tensor.matmul(
                        agg_ps[:, 1536:1537], lhsT=asn, rhs=nrm16,
                        start=first, stop=last,
                    )

                # ---- finalize image b ----
                negc = finp.tile([K, 1], F32, tag="negc")
                nc.vector.tensor_scalar_mul(negc, agg_ps[:, 1536:1537], -1.0)
                uv = vladp.tile([K, D], F32, tag="uv")
                nc.vector.scalar_tensor_tensor(
                    out=uv, in0=cen_sb, scalar=negc, in1=agg_ps[:, 0:1536],
                    op0=Alu.mult, op1=Alu.add,
                )
                sq2 = sqp.tile([P, D], BF16, tag="sq")
                r2 = finp.tile([K, 1], F32, tag="r2")
                nc.scalar.activation(sq2[:K], uv, Act.Square, accum_out=r2)
                u = finp.tile([K, 1], F32, tag="u")
                nc.scalar.sqrt(u, r2)
                um = finp.tile([K, 1], F32, tag="um")
                nc.vector.tensor_scalar_max(um, u, EPS)
                invu = finp.tile([K, 1], F32, tag="invu")
                nc.vector.reciprocal(invu, um)
                s = finp.tile([K, 1], F32, tag="s")
                nc.vector.tensor_scalar(
                    s, u, scalar1=1e30, scalar2=1.0, op0=Alu.mult, op1=Alu.min
                )
                g_ps = gpsp.tile([K, 1], F32)
                nc.tensor.matmul(g_ps, lhsT=ones64, rhs=s, start=True, stop=True)
                sg = finp.tile([K, 1], F32, tag="sg")
                nc.scalar.sqrt(sg, g_ps)
                ginv = finp.tile([K, 1], F32, tag="ginv")
                nc.vector.reciprocal(ginv, sg)
                tot = finp.tile([K, 1], F32, tag="tot")
                nc.vector.tensor_mul(tot, invu, ginv)
                vfin = vladp.tile([K, D], F32, tag="vfin")
                nc.scalar.mul(vfin, uv, tot)
                nc.sync.dma_start(
                    out=out_e.ap()[b].rearrange("(k d) -> k d", k=K), in_=vfin
                )
    _split_multi_waits(nc)
    return nc


def prep_inputs(query_descs, c_centers):
    """Host-side layout prep shared by kernel() and tests."""
    qd = np.ascontiguousarray(query_descs, dtype=np.float32)
    cc = np.ascontiguousarray(c_centers, dtype=np.float32)
    descs16 = qd.astype(NP_BF16)  # [B, N, D]
    cn = cc / np.maximum(np.linalg.norm(cc, axis=1, keepdims=True), EPS)
    cnt16 = np.ascontiguousarray(cn.T.astype(NP_BF16))  # [D, K]
    in_maps = []
    for core in range(NCORES):
        shard = np.ascontiguousarray(
            descs16[core * IMGS : (core + 1) * IMGS].reshape(IMGS * N, D)
        )
        in_maps.append({"descs": shard, "cnormt": cnt16, "centers": cc})
    return in_maps


_NC_CACHE = {}


def _get_nc():
    if "nc" not in _NC_CACHE:
        _NC_CACHE["nc"] = build_nc()
    return _NC_CACHE["nc"]


def kernel(query_descs, c_centers):
    in_maps = prep_inputs(query_descs, c_centers)
    nc = _get_nc()
    res = run_bass_kernel_spmd(nc, in_maps, core_ids=list(range(NCORES)))
    out = np.concatenate(
        [res.results[i]["out"] for i in range(NCORES)], axis=0
    )  # [B, K*D]
    return out.astype(np.float32)


# revision 10
# speedup vs baseline: 1.8562x; 1.8562x over previous
"""AnyLoc VLAD (vq_codebook) Trainium2 kernel, 8-core data parallel.

Reference computation (per image, N=1024 patches, K=64 clusters, D=1536):
  descs_n = l2norm(query_descs)                 # row-normalize descriptors
  labels  = argmax_k(descs_n . l2norm(centers)) # hard assignment
  sum_d_k = sum_{n: label=k} descs_n            # per-cluster sum
  un_vlad = sum_d_k - count_k * centers_k
  vlad    = l2norm_rows(un_vlad); flatten; l2norm

Device strategy (per core, 4 images):
  - host pre-casts descriptors to bf16 and pre-transposes/normalizes the
    tiny codebook; argmax is invariant to the descriptor's own norm so the
    sims matmul uses raw (unnormalized) descriptors
  - per 128-patch chunk: DMA load natural [128,1536]bf16, xbar-DMA
    transpose to [128d,12,128n], ACT/DVE square-accum for norms,
    12 accumulating TensorE matmuls -> sims [128,64] in PSUM,
    DVE max + (sims>=max)*inv_norm -> scaled one-hot assign [128,64]bf16,
    TensorE aggregation: assign^T @ [descs | norm] accumulated over the
    image into PSUM [64, 1537] (sum_desc and counts in one group)
  - per image: un_vlad = sum_desc - counts*centers (fp32 centers), row
    norms, ones-matmul to broadcast the global norm, final scale, DMA out.
"""

import os
import sys

import numpy as np

for _p in ("/opt/trn_rl_repo", "/root/.axon_site/_ro/trn_rl_repo"):
    if os.path.isdir(_p) and _p not in sys.path:
        sys.path.insert(0, _p)

import ml_dtypes
import bass_rust
import concourse.bass as bass
import concourse.tile as tile
from concourse import mybir
from concourse.bass_utils import run_bass_kernel_spmd

B, N, K, D = 32, 1024, 64, 1536
NCORES = 8
IMGS = B // NCORES  # images per core
P = 128
NCH = N // P   # 8 patch chunks per image
DC = D // P    # 12 feature chunks
BF16 = mybir.dt.bfloat16
F32 = mybir.dt.float32
NP_BF16 = ml_dtypes.bfloat16
Alu = mybir.AluOpType
Act = mybir.ActivationFunctionType
EPS = 1e-12


def _patch_tile_drain():
    """This walrus build only accepts ONE sync wait per instruction; Tile's
    tail drain aggregates every outstanding semaphore wait onto a single
    Drain. Spread the waits across extra per-engine drains (all still
    before the end-of-kernel barrier, so semantics are unchanged)."""
    if getattr(tile.TileContext, "_vlad_drain_patched", False):
        return
    from concourse.vector_clock import ScopedClock

    def patched(self, tick_clock, wait_clock):
        nc = self.nc
        probe = nc.sync.drain()
        wait_clock.add_sem_waits(
            probe.ins, ScopedClock({None: tick_clock.global_clock})
        )
        si = probe.ins.sync_info
        waits = list(si.on_wait) if si is not None else []
        upds = list(si.on_update) if si is not None else []
        probe.ins.sync_info = bass_rust.SyncInfo(on_wait=waits[:1], on_update=upds)
        engines = [nc.scalar, nc.vector, nc.tensor, nc.gpsimd, nc.sync]
        for i, w in enumerate(waits[1:]):
            d = engines[i % len(engines)].drain()
            dsi = d.ins.sync_info
            du = list(dsi.on_update) if dsi is not None else []
            d.ins.sync_info = bass_rust.SyncInfo(on_wait=[w], on_update=du)
        nc.all_engine_barrier()
        popped = nc._tile_sem_poison_stack.pop()
        assert popped is self._sem_poison
        nc.clear_and_free_semaphores(list(self.sems.allocated().values()))
        nc.all_engine_barrier()

    tile.TileContext._drain_and_barrier = patched
    tile.TileContext._vlad_drain_patched = True


def _split_multi_waits(nc):
    """Walrus here accepts only one sync wait per instruction. Hoist surplus
    waits onto no-op carrier instructions inserted just before, on the same
    engine (safe: same engine executes in order, so all waits still complete
    before the original instruction issues)."""
    n_new = 0
    for _bbname, bassbb in list(nc.bb_map.items()):
        bb = bassbb.bb
        out = []
        changed = False
        for ins in bb.instructions:
            si = getattr(ins, "sync_info", None)
            waits = list(si.on_wait) if si is not None else []
            if len(waits) > 1:
                changed = True
                for w in waits[:-1]:
                    n_new += 1
                    nop = mybir.InstNoOp(
                        name=f"{ins.name}-wsplit{n_new}",
                        sync_info=mybir.SyncInfo(on_wait=[w], on_update=[]),
                        bass_nofuse=True,
                        engine=ins.engine,
                    )
                    nc.register_instruction(nop)
                    out.append(nop)
                ins.sync_info = bass_rust.SyncInfo(
                    on_wait=[waits[-1]], on_update=list(si.on_update)
                )
            out.append(ins)
        if changed:
            bb.instructions = out
    return n_new


def build_nc(imgs=IMGS, nch=NCH):
    """Build the per-core Bass graph. `imgs`/`nch` shrinkable for sim tests."""
    _patch_tile_drain()
    n_rows = imgs * nch * P
    nc = bass.Bass("TRN2", target_bir_lowering=False, debug=False)
    descs_e = nc.dram_tensor("descs", [n_rows, D], BF16, kind="ExternalInput")
    # host-pretiled transpose: row r = (img*nch+ci)*128 + d_in_chunk holds
    # descsT tile data laid out as [128 d, DC, 128 n] per chunk
    descst_e = nc.dram_tensor("descst", [n_rows, D], BF16, kind="ExternalInput")
    cnt_e = nc.dram_tensor("cnormt", [D, K], BF16, kind="ExternalInput")
    cen_e = nc.dram_tensor("centers", [K, D], F32, kind="ExternalInput")
    out_e = nc.dram_tensor("out", [imgs, K * D], F32, kind="ExternalOutput")

    with tile.TileContext(nc) as tc:
        from contextlib import ExitStack

        with ExitStack() as ctx:
            consts = ctx.enter_context(tc.tile_pool(name="consts", bufs=1))
            natp = ctx.enter_context(tc.tile_pool(name="nat", bufs=4))
            tspp = ctx.enter_context(tc.tile_pool(name="tsp", bufs=4))
            sqp = ctx.enter_context(tc.tile_pool(name="sq", bufs=3))
            smallp = ctx.enter_context(tc.tile_pool(name="small", bufs=8))
            asnp = ctx.enter_context(tc.tile_pool(name="asn", bufs=4))
            vladp = ctx.enter_context(tc.tile_pool(name="vlad", bufs=2))
            finp = ctx.enter_context(tc.tile_pool(name="fin", bufs=4))
            simsp = ctx.enter_context(tc.tile_pool(name="simsps", bufs=2, space="PSUM"))
            aggp = ctx.enter_context(tc.tile_pool(name="aggps", bufs=1, space="PSUM"))
            gpsp = ctx.enter_context(tc.tile_pool(name="gps", bufs=2, space="PSUM"))

            cnt_sb = consts.tile([P, DC, K], BF16)
            nc.sync.dma_start(
                out=cnt_sb, in_=cnt_e.ap().rearrange("(c p) k -> p c k", p=P)
            )
            cen_sb = consts.tile([K, D], F32)
            nc.sync.dma_start(out=cen_sb, in_=cen_e.ap())
            ones64 = consts.tile([K, K], F32)
            nc.vector.memset(ones64, 1.0)

            for b in range(imgs):
                agg_ps = aggp.tile([K, 3 * 512 + 1], F32)
                for ci in range(nch):
                    r0 = (b * nch + ci) * P
                    nat = natp.tile([P, D], BF16)
                    nc.sync.dma_start(out=nat, in_=descs_e.ap()[r0 : r0 + P, :])
                    tsp = tspp.tile([P, DC, P], BF16)
                    nc.sync.dma_start(
                        out=tsp,
                        in_=descst_e.ap()[r0 : r0 + P, :].rearrange(
                            "p (c n) -> p c n", c=DC
                        ),
                    )

                    sq = sqp.tile([P, D], BF16, tag="sq")
                    ss = smallp.tile([P, 1], F32, tag="ss")
                    if ci % 2 == 0:
                        nc.scalar.activation(sq, nat, Act.Square, accum_out=ss)
                    else:
                        nc.vector.scalar_tensor_tensor(
                            out=sq, in0=nat, scalar=1.0, in1=nat,
                            op0=Alu.mult, op1=Alu.mult, accum_out=ss,
                        )
                    nrm16 = smallp.tile([P, 1], BF16, tag="nrm16")
                    nc.scalar.sqrt(nrm16, ss)
                    nrmx = smallp.tile([P, 1], F32, tag="nrmx")
                    nc.vector.tensor_scalar_max(nrmx, nrm16, EPS)
                    inv = smallp.tile([P, 1], F32, tag="inv")
                    nc.vector.reciprocal(inv, nrmx)

                    sims = simsp.tile([P, K], F32)
                    for d in range(DC):
                        nc.tensor.matmul(
                            sims, lhsT=tsp[:, d, :], rhs=cnt_sb[:, d, :],
                            start=(d == 0), stop=(d == DC - 1),
                        )
                    mx = smallp.tile([P, 1], F32, tag="mx")
                    nc.vector.tensor_reduce(mx, sims, axis=mybir.AxisListType.X, op=Alu.max)
                    asn = asnp.tile([P, K], BF16)
                    nc.vector.tensor_scalar(
                        asn, sims, scalar1=mx, scalar2=inv, op0=Alu.is_ge, op1=Alu.mult
                    )

                    first, last = ci == 0, ci == nch - 1
                    for j in range(3):
                        nc.tensor.matmul(
                            agg_ps[:, j * 512 : (j + 1) * 512],
                            lhsT=asn, rhs=nat[:, j * 512 : (j + 1) * 512],
                            start=first, stop=last,
                        )
                    nc.tensor.matmul(
                        agg_ps[:, 1536:1537], lhsT=asn, rhs=nrm16,
                        start=first, stop=last,
                    )

                # ---- finalize image b ----
                negc = finp.tile([K, 1], F32, tag="negc")
                nc.vector.tensor_scalar_mul(negc, agg_ps[:, 1536:1537], -1.0)
                uv = vladp.tile([K, D], F32, tag="uv")
                nc.vector.scalar_tensor_tensor(
                    out=uv, in0=cen_sb, scalar=negc, in1=agg_ps[:, 0:1536],
                    op0=Alu.mult, op1=Alu.add,
                )
                sq2 = sqp.tile([P, D], BF16, tag="sq")
                r2 = finp.tile([K, 1], F32, tag="r2")
                nc.scalar.activation(sq2[:K], uv, Act.Square, accum_out=r2)
                u = finp.tile([K, 1], F32, tag="u")
                nc.scalar.sqrt(u, r2)
                um = finp.tile([K, 1], F32, tag="um")
                nc.vector.tensor_scalar_max(um, u, EPS)
                invu = finp.tile([K, 1], F32, tag="invu")
                nc.vector.reciprocal(invu, um)
                s = finp.tile([K, 1], F32, tag="s")
                nc.vector.tensor_scalar(
                    s, u, scalar1=1e30, scalar2=1.0, op0=Alu.mult, op1=Alu.min
                )
                g_ps = gpsp.tile([K, 1], F32)
                nc.tensor.matmul(g_ps, lhsT=ones64, rhs=s, start=True, stop=True)
                sg = finp.tile([K, 1], F32, tag="sg")
                nc.scalar.sqrt(sg, g_ps)
                ginv = finp.tile([K, 1], F32, tag="ginv")
                nc.vector.reciprocal(ginv, sg)
                tot = finp.tile([K, 1], F32, tag="tot")
                nc.vector.tensor_mul(tot, invu, ginv)
                vfin = vladp.tile([K, D], F32, tag="vfin")
                nc.scalar.mul(vfin, uv, tot)
                nc.sync.dma_start(
                    out=out_e.ap()[b].rearrange("(k d) -> k d", k=K), in_=vfin
                )
    _split_multi_waits(nc)
    return nc


def prep_inputs(query_descs, c_centers):
    """Host-side layout prep shared by kernel() and tests."""
    qd = np.ascontiguousarray(query_descs, dtype=np.float32)
    cc = np.ascontiguousarray(c_centers, dtype=np.float32)
    descs16 = qd.astype(NP_BF16)  # [B, N, D]
    cn = cc / np.maximum(np.linalg.norm(cc, axis=1, keepdims=True), EPS)
    cnt16 = np.ascontiguousarray(cn.T.astype(NP_BF16))  # [D, K]
    in_maps = []
    for core in range(NCORES):
        sh = descs16[core * IMGS : (core + 1) * IMGS]  # [IMGS, N, D]
        shard = np.ascontiguousarray(sh.reshape(IMGS * N, D))
        # pretiled transpose: row (b*NCH+ci)*128+p holds [DC, 128n] with
        # element (p, c, n) = descs[b, ci*128+n, c*128+p]
        sht = np.ascontiguousarray(
            sh.reshape(IMGS, NCH, P, DC, P).transpose(0, 1, 4, 3, 2)
        ).reshape(IMGS * N, D)
        in_maps.append(
            {"descs": shard, "descst": sht, "cnormt": cnt16, "centers": cc}
        )
    return in_maps


_NC_CACHE = {}


def _get_nc():
    if "nc" not in _NC_CACHE:
        _NC_CACHE["nc"] = build_nc()
    return _NC_CACHE["nc"]


def kernel(query_descs, c_centers):
    in_maps = prep_inputs(query_descs, c_centers)
    nc = _get_nc()
    res = run_bass_kernel_spmd(nc, in_maps, core_ids=list(range(NCORES)))
    out = np.concatenate(
        [res.results[i]["out"] for i in range(NCORES)], axis=0
    )  # [B, K*D]
    return out.astype(np.float32)


# revision 11
# speedup vs baseline: 2.1951x; 1.1826x over previous
"""AnyLoc VLAD (vq_codebook) Trainium2 kernel, 8-core data parallel.

Reference computation (per image, N=1024 patches, K=64 clusters, D=1536):
  descs_n = l2norm(query_descs)                 # row-normalize descriptors
  labels  = argmax_k(descs_n . l2norm(centers)) # hard assignment
  sum_d_k = sum_{n: label=k} descs_n            # per-cluster sum
  un_vlad = sum_d_k - count_k * centers_k
  vlad    = l2norm_rows(un_vlad); flatten; l2norm

Device strategy (per core, 4 images):
  - host pre-casts descriptors to bf16 and pre-transposes/normalizes the
    tiny codebook; argmax is invariant to the descriptor's own norm so the
    sims matmul uses raw (unnormalized) descriptors
  - per 128-patch chunk: DMA load natural [128,1536]bf16, xbar-DMA
    transpose to [128d,12,128n], ACT/DVE square-accum for norms,
    12 accumulating TensorE matmuls -> sims [128,64] in PSUM,
    DVE max + (sims>=max)*inv_norm -> scaled one-hot assign [128,64]bf16,
    TensorE aggregation: assign^T @ [descs | norm] accumulated over the
    image into PSUM [64, 1537] (sum_desc and counts in one group)
  - per image: un_vlad = sum_desc - counts*centers (fp32 centers), row
    norms, ones-matmul to broadcast the global norm, final scale, DMA out.
"""

import os
import sys

import numpy as np

for _p in ("/opt/trn_rl_repo", "/root/.axon_site/_ro/trn_rl_repo"):
    if os.path.isdir(_p) and _p not in sys.path:
        sys.path.insert(0, _p)

import ml_dtypes
import bass_rust
import concourse.bass as bass
import concourse.tile as tile
from concourse import mybir
from concourse.bass_utils import run_bass_kernel_spmd

B, N, K, D = 32, 1024, 64, 1536
NCORES = 8
IMGS = B // NCORES  # images per core
P = 128
NCH = N // P   # 8 patch chunks per image
DC = D // P    # 12 feature chunks
BF16 = mybir.dt.bfloat16
FP8 = mybir.dt.float8e4
F32 = mybir.dt.float32
NP_BF16 = ml_dtypes.bfloat16
NP_FP8 = ml_dtypes.float8_e4m3
Alu = mybir.AluOpType
Act = mybir.ActivationFunctionType
EPS = 1e-12


def _patch_tile_drain():
    """This walrus build only accepts ONE sync wait per instruction; Tile's
    tail drain aggregates every outstanding semaphore wait onto a single
    Drain. Spread the waits across extra per-engine drains (all still
    before the end-of-kernel barrier, so semantics are unchanged)."""
    if getattr(tile.TileContext, "_vlad_drain_patched", False):
        return
    from concourse.vector_clock import ScopedClock

    def patched(self, tick_clock, wait_clock):
        nc = self.nc
        probe = nc.sync.drain()
        wait_clock.add_sem_waits(
            probe.ins, ScopedClock({None: tick_clock.global_clock})
        )
        si = probe.ins.sync_info
        waits = list(si.on_wait) if si is not None else []
        upds = list(si.on_update) if si is not None else []
        probe.ins.sync_info = bass_rust.SyncInfo(on_wait=waits[:1], on_update=upds)
        engines = [nc.scalar, nc.vector, nc.tensor, nc.gpsimd, nc.sync]
        for i, w in enumerate(waits[1:]):
            d = engines[i % len(engines)].drain()
            dsi = d.ins.sync_info
            du = list(dsi.on_update) if dsi is not None else []
            d.ins.sync_info = bass_rust.SyncInfo(on_wait=[w], on_update=du)
        nc.all_engine_barrier()
        popped = nc._tile_sem_poison_stack.pop()
        assert popped is self._sem_poison
        nc.clear_and_free_semaphores(list(self.sems.allocated().values()))
        nc.all_engine_barrier()

    tile.TileContext._drain_and_barrier = patched
    tile.TileContext._vlad_drain_patched = True


def _split_multi_waits(nc):
    """Walrus here accepts only one sync wait per instruction. Hoist surplus
    waits onto no-op carrier instructions inserted just before, on the same
    engine (safe: same engine executes in order, so all waits still complete
    before the original instruction issues)."""
    n_new = 0
    for _bbname, bassbb in list(nc.bb_map.items()):
        bb = bassbb.bb
        out = []
        changed = False
        for ins in bb.instructions:
            si = getattr(ins, "sync_info", None)
            waits = list(si.on_wait) if si is not None else []
            if len(waits) > 1:
                changed = True
                for w in waits[:-1]:
                    n_new += 1
                    nop = mybir.InstNoOp(
                        name=f"{ins.name}-wsplit{n_new}",
                        sync_info=mybir.SyncInfo(on_wait=[w], on_update=[]),
                        bass_nofuse=True,
                        engine=ins.engine,
                    )
                    nc.register_instruction(nop)
                    out.append(nop)
                ins.sync_info = bass_rust.SyncInfo(
                    on_wait=[waits[-1]], on_update=list(si.on_update)
                )
            out.append(ins)
        if changed:
            bb.instructions = out
    return n_new


def build_nc(imgs=IMGS, nch=NCH):
    """Build the per-core Bass graph. `imgs`/`nch` shrinkable for sim tests."""
    _patch_tile_drain()
    n_rows = imgs * nch * P
    nc = bass.Bass("TRN2", target_bir_lowering=False, debug=False)
    descs_e = nc.dram_tensor("descs", [n_rows, D], FP8, kind="ExternalInput")
    # host-pretiled transpose: row r = (img*nch+ci)*128 + d_in_chunk holds
    # descsT tile data laid out as [128 d, DC, 128 n] per chunk
    descst_e = nc.dram_tensor("descst", [n_rows, D], FP8, kind="ExternalInput")
    cnt_e = nc.dram_tensor("cnormt", [D, K], FP8, kind="ExternalInput")
    cen_e = nc.dram_tensor("centers", [K, D], F32, kind="ExternalInput")
    out_e = nc.dram_tensor("out", [imgs, K * D], F32, kind="ExternalOutput")

    with tile.TileContext(nc) as tc:
        from contextlib import ExitStack

        with ExitStack() as ctx:
            consts = ctx.enter_context(tc.tile_pool(name="consts", bufs=1))
            natp = ctx.enter_context(tc.tile_pool(name="nat", bufs=4))
            tspp = ctx.enter_context(tc.tile_pool(name="tsp", bufs=4))
            sqp = ctx.enter_context(tc.tile_pool(name="sq", bufs=3))
            smallp = ctx.enter_context(tc.tile_pool(name="small", bufs=8))
            asnp = ctx.enter_context(tc.tile_pool(name="asn", bufs=4))
            vladp = ctx.enter_context(tc.tile_pool(name="vlad", bufs=2))
            finp = ctx.enter_context(tc.tile_pool(name="fin", bufs=4))
            simsp = ctx.enter_context(tc.tile_pool(name="simsps", bufs=2, space="PSUM"))
            aggp = ctx.enter_context(tc.tile_pool(name="aggps", bufs=1, space="PSUM"))
            gpsp = ctx.enter_context(tc.tile_pool(name="gps", bufs=2, space="PSUM"))

            cnt_sb = consts.tile([P, DC, K], FP8)
            nc.sync.dma_start(
                out=cnt_sb, in_=cnt_e.ap().rearrange("(c p) k -> p c k", p=P)
            )
            cen_sb = consts.tile([K, D], F32)
            nc.sync.dma_start(out=cen_sb, in_=cen_e.ap())
            ones64 = consts.tile([K, K], F32)
            nc.vector.memset(ones64, 1.0)

            for b in range(imgs):
                agg_ps = aggp.tile([K, 3 * 512 + 1], F32)
                for ci in range(nch):
                    r0 = (b * nch + ci) * P
                    nat = natp.tile([P, D], FP8)
                    nc.sync.dma_start(out=nat, in_=descs_e.ap()[r0 : r0 + P, :])
                    tsp = tspp.tile([P, DC, P], FP8)
                    nc.sync.dma_start(
                        out=tsp,
                        in_=descst_e.ap()[r0 : r0 + P, :].rearrange(
                            "p (c n) -> p c n", c=DC
                        ),
                    )

                    sq = sqp.tile([P, D], FP8, tag="sq")
                    ss = smallp.tile([P, 1], F32, tag="ss")
                    nc.vector.scalar_tensor_tensor(
                        out=sq, in0=nat, scalar=1.0, in1=nat,
                        op0=Alu.mult, op1=Alu.mult, accum_out=ss,
                    )
                    # nrmq = sqrt(ss)/64 in fp8 (the 64x cancels against inv64
                    # in every downstream product; un_vlad is scale-invariant)
                    nrmq = smallp.tile([P, 1], FP8, tag="nrmq")
                    nc.scalar.activation(
                        nrmq, ss, Act.Sqrt, scale=1.0 / 4096.0
                    )
                    inv = smallp.tile([P, 1], F32, tag="inv")
                    nc.vector.reciprocal(inv, nrmq)

                    sims = simsp.tile([P, K], F32)
                    for d in range(DC):
                        nc.tensor.matmul(
                            sims, lhsT=tsp[:, d, :], rhs=cnt_sb[:, d, :],
                            start=(d == 0), stop=(d == DC - 1),
                        )
                    mx = smallp.tile([P, 1], F32, tag="mx")
                    nc.vector.tensor_reduce(mx, sims, axis=mybir.AxisListType.X, op=Alu.max)
                    asn = asnp.tile([P, K], FP8)
                    nc.vector.tensor_scalar(
                        asn, sims, scalar1=mx, scalar2=inv, op0=Alu.is_ge, op1=Alu.mult
                    )

                    first, last = ci == 0, ci == nch - 1
                    for j in range(3):
                        nc.tensor.matmul(
                            agg_ps[:, j * 512 : (j + 1) * 512],
                            lhsT=asn, rhs=nat[:, j * 512 : (j + 1) * 512],
                            start=first, stop=last,
                        )
                    nc.tensor.matmul(
                        agg_ps[:, 1536:1537], lhsT=asn, rhs=nrmq,
                        start=first, stop=last,
                    )

                # ---- finalize image b ----
                negc = finp.tile([K, 1], F32, tag="negc")
                nc.vector.tensor_scalar_mul(negc, agg_ps[:, 1536:1537], -64.0)
                uv = vladp.tile([K, D], F32, tag="uv")
                nc.vector.scalar_tensor_tensor(
                    out=uv, in0=cen_sb, scalar=negc, in1=agg_ps[:, 0:1536],
                    op0=Alu.mult, op1=Alu.add,
                )
                sq2 = sqp.tile([P, D], FP8, tag="sq")
                r2 = finp.tile([K, 1], F32, tag="r2")
                nc.scalar.activation(sq2[:K], uv, Act.Square, accum_out=r2)
                u = finp.tile([K, 1], F32, tag="u")
                nc.scalar.sqrt(u, r2)
                um = finp.tile([K, 1], F32, tag="um")
                nc.vector.tensor_scalar_max(um, u, EPS)
                invu = finp.tile([K, 1], F32, tag="invu")
                nc.vector.reciprocal(invu, um)
                s = finp.tile([K, 1], F32, tag="s")
                nc.vector.tensor_scalar(
                    s, u, scalar1=1e30, scalar2=1.0, op0=Alu.mult, op1=Alu.min
                )
                g_ps = gpsp.tile([K, 1], F32)
                nc.tensor.matmul(g_ps, lhsT=ones64, rhs=s, start=True, stop=True)
                sg = finp.tile([K, 1], F32, tag="sg")
                nc.scalar.sqrt(sg, g_ps)
                ginv = finp.tile([K, 1], F32, tag="ginv")
                nc.vector.reciprocal(ginv, sg)
                tot = finp.tile([K, 1], F32, tag="tot")
                nc.vector.tensor_mul(tot, invu, ginv)
                vfin = vladp.tile([K, D], F32, tag="vfin")
                nc.scalar.mul(vfin, uv, tot)
                nc.sync.dma_start(
                    out=out_e.ap()[b].rearrange("(k d) -> k d", k=K), in_=vfin
                )
    _split_multi_waits(nc)
    return nc


def prep_inputs(query_descs, c_centers):
    """Host-side layout prep shared by kernel() and tests."""
    qd = np.ascontiguousarray(query_descs, dtype=np.float32)
    cc = np.ascontiguousarray(c_centers, dtype=np.float32)
    descs16 = qd.astype(NP_FP8)  # [B, N, D]
    cn = cc / np.maximum(np.linalg.norm(cc, axis=1, keepdims=True), EPS)
    # x64 so the fp8 codebook lands in e4m3's sweet spot; argmax and the
    # max-compare are invariant to a uniform positive scale on sims
    cnt16 = np.ascontiguousarray((cn.T * 64.0).astype(NP_FP8))  # [D, K]
    in_maps = []
    for core in range(NCORES):
        sh = descs16[core * IMGS : (core + 1) * IMGS]  # [IMGS, N, D]
        shard = np.ascontiguousarray(sh.reshape(IMGS * N, D))
        # pretiled transpose: row (b*NCH+ci)*128+p holds [DC, 128n] with
        # element (p, c, n) = descs[b, ci*128+n, c*128+p]
        sht = np.ascontiguousarray(
            sh.reshape(IMGS, NCH, P, DC, P).transpose(0, 1, 4, 3, 2)
        ).reshape(IMGS * N, D)
        in_maps.append(
            {"descs": shard, "descst": sht, "cnormt": cnt16, "centers": cc}
        )
    return in_maps


_NC_CACHE = {}


def _get_nc():
    if "nc" not in _NC_CACHE:
        _NC_CACHE["nc"] = build_nc()
    return _NC_CACHE["nc"]


def kernel(query_descs, c_centers):
    in_maps = prep_inputs(query_descs, c_centers)
    nc = _get_nc()
    res = run_bass_kernel_spmd(nc, in_maps, core_ids=list(range(NCORES)))
    out = np.concatenate(
        [res.results[i]["out"] for i in range(NCORES)], axis=0
    )  # [B, K*D]
    return out.astype(np.float32)
